# revision 1
# baseline (speedup 1.0000x reference)
"""nn_MultiHeadAttention TRN2 kernel: 8-core tensor-parallel (2 heads/core).

Self-contained: builds and compiles the Bass/Tile SPMD program on first call,
shards the full inputs per-core on the host, runs via run_bass_kernel_spmd,
and concatenates the per-core sequence-block outputs into the full output.

Algorithm (per core, 2 heads of 16, head_dim 64, S=4096, D=1024):
  - feature-major layout: xT [D,S]; q/k projected with RoPE-permuted,
    transposed weight shards so rotary becomes a contiguous split-half
    rotation; v seq-major with a ones column (softmax denominator).
  - flash attention on transposed score tiles scoresT[j,i]: PE matmuls
    (heads row-packed), causal mask added as a -400 triangle on diagonal
    tiles (DVE), exp on ScalarE grouped 3 key-tiles per instruction,
    PV accumulates outT[65,512] in PSUM (row 64 = denominator).
  - normalize via DVE reciprocal + K=1 matmul partition-broadcast.
  - AllToAll re-shards from head-split to sequence-split; final projection
    against full Wo.T; each core emits out[512, 1024] f32.
"""

from contextlib import ExitStack

import numpy as np
import ml_dtypes

import concourse.tile as tile
from concourse import bacc, mybir
from concourse.bass_utils import run_bass_kernel_spmd

F32 = mybir.dt.float32
BF16 = mybir.dt.bfloat16

S = 4096
D = 1024
HD = 64
N_CORES = 8
KT = 128
BQ = 512


def _build():
    CHUNK = S // N_CORES
    n_qb = S // BQ
    bq = BQ
    n_kt = S // KT
    n_ft = D // 128

    nc = bacc.Bacc("TRN2", target_bir_lowering=False, debug=False, num_devices=N_CORES)

    xT = nc.dram_tensor("xT", [D, S], BF16, kind="ExternalInput")
    wq = nc.dram_tensor("wq", [D, 128], BF16, kind="ExternalInput")
    wk = nc.dram_tensor("wk", [D, 128], BF16, kind="ExternalInput")
    wv = nc.dram_tensor("wv", [D, 128], BF16, kind="ExternalInput")
    wo = nc.dram_tensor("wo", [D, D], BF16, kind="ExternalInput")
    cosP = nc.dram_tensor("cosP", [128, S], BF16, kind="ExternalInput")
    sinN = nc.dram_tensor("sinN", [128, S], BF16, kind="ExternalInput")
    lu = nc.dram_tensor("lu", [128, 128], BF16, kind="ExternalInput")
    out = nc.dram_tensor("out", [CHUNK, D], F32, kind="ExternalOutput")

    a2a_in = nc.dram_tensor("a2a_in", [N_CORES * 128, CHUNK], BF16)
    a2a_out = nc.dram_tensor("a2a_out", [N_CORES * 128, CHUNK], BF16)

    with tile.TileContext(nc) as tc, ExitStack() as ctx:
        sb = ctx.enter_context(tc.tile_pool(name="sb", bufs=1))
        xt_s = [sb.tile([128, S], BF16, tag=f"xt{t}", name=f"xt{t}") for t in range(n_ft)]
        wq_s = sb.tile([128, n_ft * 128], BF16, tag="wq", name="wq_s")
        wk_s = sb.tile([128, n_ft * 128], BF16, tag="wk", name="wk_s")
        wv_s = sb.tile([128, n_ft * 128], BF16, tag="wv", name="wv_s")
        wo_s = [sb.tile([128, D], BF16, tag=f"wo{t}", name=f"wo_s{t}") for t in range(n_ft)]
        cos_s = sb.tile([128, S], BF16, tag="cos", name="cos_s")
        sin_s = sb.tile([128, S], BF16, tag="sin", name="sin_s")
        lu_s = sb.tile([128, 128], BF16, tag="lu", name="lu_s")
        qA = sb.tile([128, S], BF16, tag="qA", name="qA")
        kA = sb.tile([128, S], BF16, tag="kA", name="kA")
        qB = sb.tile([128, S], BF16, tag="qB", name="qB")
        kB = sb.tile([128, S], BF16, tag="kB", name="kB")
        qT = sb.tile([128, S], BF16, tag="qT", name="qT")
        kT_ = sb.tile([128, S], BF16, tag="kT", name="kT_")
        v_aug = sb.tile([128, n_kt * 130], BF16, tag="vaug", name="v_aug")
        attnT = sb.tile([128, S], BF16, tag="attnT", name="attnT")
        aT = [sb.tile([128, CHUNK], BF16, tag=f"aT{t}", name=f"aT{t}") for t in range(n_ft)]

        for t in range(n_ft):
            nc.sync.dma_start(xt_s[t][:], xT[128 * t : 128 * (t + 1), :])
            nc.sync.dma_start(wq_s[:, 128 * t : 128 * (t + 1)], wq[128 * t : 128 * (t + 1), :])
            nc.sync.dma_start(wk_s[:, 128 * t : 128 * (t + 1)], wk[128 * t : 128 * (t + 1), :])
            nc.sync.dma_start(wv_s[:, 128 * t : 128 * (t + 1)], wv[128 * t : 128 * (t + 1), :])
            nc.sync.dma_start(wo_s[t][:], wo[128 * t : 128 * (t + 1), :])
        nc.sync.dma_start(cos_s[:], cosP[:, :])
        nc.sync.dma_start(sin_s[:], sinN[:, :])
        nc.sync.dma_start(lu_s[:], lu[:, :])

        psc = ctx.enter_context(tc.tile_pool(name="psc", bufs=2, space="PSUM"))
        ppv = ctx.enter_context(tc.tile_pool(name="ppv", bufs=2, space="PSUM"))

        # projections
        for w_s, dst in ((wq_s, qA), (wk_s, kA)):
            for nb in range(S // bq):
                p = psc.tile([128, bq], F32, tag="sc", name="p_qk")
                for t in range(n_ft):
                    nc.tensor.matmul(
                        p[:],
                        w_s[:, 128 * t : 128 * (t + 1)],
                        xt_s[t][:, bq * nb : bq * (nb + 1)],
                        start=(t == 0),
                        stop=(t == n_ft - 1),
                    )
                nc.scalar.copy(dst[:, bq * nb : bq * (nb + 1)], p[:])
        for st in range(n_kt):
            p = ppv.tile([128, 128], F32, tag="pv", name="p_v")
            for t in range(n_ft):
                nc.tensor.matmul(
                    p[:],
                    xt_s[t][:, 128 * st : 128 * (st + 1)],
                    wv_s[:, 128 * t : 128 * (t + 1)],
                    start=(t == 0),
                    stop=(t == n_ft - 1),
                )
            base = 130 * st
            nc.vector.tensor_copy(v_aug[:, base : base + 64], p[:, 0:64])
            nc.vector.tensor_copy(v_aug[:, base + 65 : base + 129], p[:, 64:128])
            nc.vector.memset(v_aug[:, base + 64 : base + 65], 1.0)
            nc.vector.memset(v_aug[:, base + 129 : base + 130], 1.0)

        # RoPE
        for A, B in ((qA, qB), (kA, kB)):
            for h in range(2):
                b0 = 64 * h
                nc.sync.dma_start(B[b0 : b0 + 32, :], A[b0 + 32 : b0 + 64, :])
                nc.sync.dma_start(B[b0 + 32 : b0 + 64, :], A[b0 : b0 + 32, :])
        for A, B, Rt in ((qA, qB, qT), (kA, kB, kT_)):
            nc.vector.tensor_mul(Rt[:], A[:], cos_s[:])
            nc.vector.tensor_mul(B[:], B[:], sin_s[:])
            nc.vector.tensor_add(Rt[:], Rt[:], B[:])

        # attention
        ones_col = sb.tile([1, 64], F32, tag="ones_col", name="ones_col")
        nc.vector.memset(ones_col[:], 1.0)
        GROUP = 3
        for Q in range(n_qb):
            q0 = bq * Q
            n_jt = min((q0 + bq) // KT, n_kt)
            outT = {}
            for h in range(2):
                outT[h] = ppv.tile([65, bq], F32, tag="pv", name=f"outT_h{h}")
            for h in range(2):
                hb = 64 * h
                jts = list(range(n_jt))
                groups = [jts[i : i + GROUP] for i in range(0, n_jt, GROUP)]
                for g in groups:
                    sc = psc.tile([128, len(g) * bq], F32, tag="sc", name="sc_g")
                    for idx, jt in enumerate(g):
                        nc.tensor.matmul(
                            sc[:, bq * idx : bq * (idx + 1)],
                            kT_[hb : hb + 64, KT * jt : KT * (jt + 1)],
                            qT[hb : hb + 64, q0 : q0 + bq],
                            start=True,
                            stop=True,
                        )
                        if KT * jt >= q0:
                            trim = KT * jt - q0
                            nc.vector.tensor_add(
                                sc[:, bq * idx + trim : bq * idx + trim + 128],
                                sc[:, bq * idx + trim : bq * idx + trim + 128],
                                lu_s[:],
                            )
                    expT = sb.tile([128, GROUP * bq], BF16, tag="expT", name="expT", bufs=2)
                    nc.scalar.activation(
                        expT[:, 0 : len(g) * bq],
                        sc[:],
                        mybir.ActivationFunctionType.Exp,
                        scale=0.125,
                    )
                    for idx, jt in enumerate(g):
                        trim = max(0, KT * jt - q0)
                        nc.tensor.matmul(
                            outT[h][:, trim:bq],
                            v_aug[:, 130 * jt : 130 * jt + 65]
                            if h == 0
                            else v_aug[:, 130 * jt + 65 : 130 * jt + 130],
                            expT[:, bq * idx + trim : bq * (idx + 1)],
                            start=(jt == 0),
                            stop=(jt == n_jt - 1),
                        )
                den_r = sb.tile([1, bq], F32, tag="den", name="den_r")
                nc.vector.reciprocal(den_r[:], outT[h][64:65, :])
                bc = psc.tile([64, bq], F32, tag="sc", name="bc")
                nc.tensor.matmul(bc[:], ones_col[:], den_r[:], start=True, stop=True)
                bc_sb = sb.tile([64, bq], F32, tag="bc_sb", name="bc_sb")
                nc.vector.tensor_copy(bc_sb[:], bc[:])
                nc.vector.tensor_mul(
                    attnT[hb : hb + 64, q0 : q0 + bq], outT[h][0:64, :], bc_sb[:]
                )

        # all-to-all: head-split -> sequence-split
        for j in range(N_CORES):
            nc.sync.dma_start(
                a2a_in[128 * j : 128 * (j + 1), :], attnT[:, CHUNK * j : CHUNK * (j + 1)]
            )
        nc.gpsimd.collective_compute(
            "AllToAll",
            mybir.AluOpType.bypass,
            replica_groups=[list(range(N_CORES))],
            ins=[a2a_in.ap().opt()],
            outs=[a2a_out.ap().opt()],
        )
        for t in range(n_ft):
            nc.sync.dma_start(aT[t][:], a2a_out[128 * t : 128 * (t + 1), :])

        # output projection
        for it in range(CHUNK // 128):
            for oh in range(D // 512):
                p = psc.tile([128, 512], F32, tag="sc", name="p_o")
                for t in range(n_ft):
                    nc.tensor.matmul(
                        p[:],
                        aT[t][:, 128 * it : 128 * (it + 1)],
                        wo_s[t][:, 512 * oh : 512 * (oh + 1)],
                        start=(t == 0),
                        stop=(t == n_ft - 1),
                    )
                ot = sb.tile([128, 512], F32, tag="oflush", name="ot")
                nc.scalar.copy(ot[:], p[:])
                nc.sync.dma_start(
                    out[128 * it : 128 * (it + 1), 512 * oh : 512 * (oh + 1)], ot[:]
                )

    nc.compile()
    return nc


def _host_prep(x, Wq, Wk, Wv, Wo):
    bf = ml_dtypes.bfloat16
    perm = np.empty(HD, dtype=np.int64)
    perm[:32] = np.arange(0, 64, 2)
    perm[32:] = np.arange(1, 64, 2)

    inv_freq = 1.0 / (10000.0 ** (np.arange(0, HD, 2, dtype=np.float32) / HD))
    fr = np.outer(np.arange(S, dtype=np.float32), inv_freq)
    cosA = np.cos(fr).T
    sinA = np.sin(fr).T
    cosP = np.tile(np.concatenate([cosA, cosA], 0), (2, 1)).astype(bf)
    sinN = np.tile(np.concatenate([-sinA, sinA], 0), (2, 1)).astype(bf)
    lu = np.tril(np.full((128, 128), -400.0, np.float32), k=-1).astype(bf)

    xT = np.ascontiguousarray(x.reshape(S, D).T).astype(bf)
    woT = np.ascontiguousarray(np.asarray(Wo, np.float32).T).astype(bf)

    in_maps = []
    for c in range(N_CORES):
        rows = np.concatenate([128 * c + 64 * h + perm for h in range(2)])
        in_maps.append(
            {
                "xT": xT,
                "wq": np.ascontiguousarray(np.asarray(Wq, np.float32)[rows].T).astype(bf),
                "wk": np.ascontiguousarray(np.asarray(Wk, np.float32)[rows].T).astype(bf),
                "wv": np.ascontiguousarray(
                    np.asarray(Wv, np.float32)[128 * c : 128 * (c + 1)].T
                ).astype(bf),
                "wo": woT,
                "cosP": cosP,
                "sinN": sinN,
                "lu": lu,
            }
        )
    return in_maps


_NC_CACHE = None


def kernel(x, Wq, Wk, Wv, Wo):
    global _NC_CACHE
    if _NC_CACHE is None:
        _NC_CACHE = _build()
    nc = _NC_CACHE
    in_maps = _host_prep(
        np.asarray(x, np.float32),
        np.asarray(Wq, np.float32),
        np.asarray(Wk, np.float32),
        np.asarray(Wv, np.float32),
        np.asarray(Wo, np.float32),
    )
    res = run_bass_kernel_spmd(nc, in_maps, core_ids=list(range(N_CORES)))
    full = np.concatenate([res.results[c]["out"] for c in range(N_CORES)], axis=0)
    return full.reshape(1, S, D).astype(np.float32)
